# revision 3
# baseline (speedup 1.0000x reference)
import sys

import numpy as np

N_NODES = 19
HID = 128
HEADS = 8
DH = HID // HEADS
NLAYERS = 3
EPS = 1e-5
BATCH = 1024
IN_DIM = 3000
N_CORES = 8


def _layer_norm(x, g, b):
    m = x.mean(axis=-1, keepdims=True)
    v = ((x - m) ** 2).mean(axis=-1, keepdims=True)
    return (x - m) / np.sqrt(v + EPS) * g + b


def _softmax(x, axis):
    x = x - x.max(axis=axis, keepdims=True)
    e = np.exp(x)
    return e / e.sum(axis=axis, keepdims=True)


def _embed_host(node_features, emb_h_w, bias_full):
    # h[b,n,:] = node_features[b,:,n] @ emb_h_w + bias_full[n,:]
    x = np.ascontiguousarray(node_features.transpose(0, 2, 1))
    return x @ emb_h_w + bias_full[None, :, :]


def _embed_device(node_features, emb_h_w, bias_full):
    """Embedding matmul on 8 NeuronCores via bass: shard batch, contract 3000-dim."""
    import sys
    for p in ("/opt/trn_rl_repo",):
        if p not in sys.path:
            sys.path.insert(0, p)
    import concourse.bass as bass
    import concourse.tile as tile
    from concourse import mybir
    from concourse import bass_utils

    B = node_features.shape[0]
    BPC = B // N_CORES          # 128 batches per core
    GB = 16                     # batch group per matmul set
    NG = BPC // GB              # 8 groups
    RCH = 24                    # contraction split: 3000 = 24 * 125
    P = 125

    nc = bass.Bass()
    nf = nc.dram_tensor("nf", [BPC, IN_DIM, N_NODES], mybir.dt.float32,
                        kind="ExternalInput")
    w = nc.dram_tensor("w", [RCH, P, HID], mybir.dt.float32, kind="ExternalInput")
    bias = nc.dram_tensor("bias", [HID, N_NODES], mybir.dt.float32,
                          kind="ExternalInput")
    out = nc.dram_tensor("out", [HID, BPC, N_NODES], mybir.dt.float32,
                         kind="ExternalOutput")

    with tile.TileContext(nc) as tc:
        with tc.tile_pool(name="consts", bufs=1) as consts, \
             tc.tile_pool(name="nfp", bufs=3) as nfp, \
             tc.tile_pool(name="ps", bufs=4, space="PSUM") as ps, \
             tc.tile_pool(name="outp", bufs=3) as outp:
            wt = consts.tile([P, RCH, HID], mybir.dt.float32)
            nc.sync.dma_start(out=wt, in_=w.rearrange("r p m -> p r m"))
            bt = consts.tile([HID, N_NODES], mybir.dt.float32)
            nc.sync.dma_start(out=bt, in_=bias)
            for g in range(NG):
                nft = nfp.tile([P, GB, RCH, N_NODES], mybir.dt.float32)
                # partition p holds rows [p*24,(p+1)*24) of each [3000,19] slab
                nc.sync.dma_start(
                    out=nft,
                    in_=nf[g * GB:(g + 1) * GB].rearrange("b (p r) n -> p b r n", p=P),
                )
                acc = ps.tile([HID, GB * N_NODES], mybir.dt.float32)
                for r in range(RCH):
                    nc.tensor.matmul(
                        acc,
                        lhsT=wt[:, r, :].bitcast(mybir.dt.float32r),
                        rhs=nft[:, :, r, :].rearrange("p b n -> p (b n)").bitcast(
                            mybir.dt.float32r),
                        start=(r == 0), stop=(r == RCH - 1),
                    )
                ot = outp.tile([HID, GB, N_NODES], mybir.dt.float32)
                nc.vector.tensor_add(
                    ot, acc.rearrange("h (b n) -> h b n", b=GB),
                    bt[:, None, :].broadcast_to([HID, GB, N_NODES]))
                nc.sync.dma_start(out=out[:, g * GB:(g + 1) * GB, :], in_=ot)

    w_re = np.ascontiguousarray(
        emb_h_w.reshape(P, RCH, HID).transpose(1, 0, 2))  # [24,125,128]
    in_maps = []
    for c in range(N_CORES):
        in_maps.append({
            "nf": np.ascontiguousarray(node_features[c * BPC:(c + 1) * BPC]),
            "w": w_re,
            "bias": np.ascontiguousarray(bias_full.T),
        })
    res = bass_utils.run_bass_kernel_spmd(nc, in_maps, core_ids=list(range(N_CORES)))
    hs = [r["out"].transpose(1, 2, 0) for r in res.results]  # [BPC,19,128]
    return np.concatenate(hs, axis=0)


def kernel(node_features, pe, edge_index,
           emb_h_w, emb_h_b, emb_pe_w, emb_pe_b,
           wq_w, wq_b, wk_w, wk_b, wv_w, wv_b, wo_w, wo_b,
           ln1_g, ln1_b, lin1_w, lin1_b, lin2_w, lin2_b, ln2_g, ln2_b,
           mlp_w0, mlp_b0, mlp_w1, mlp_b1, mlp_w2, mlp_b2):
    f32 = np.float32
    node_features = np.asarray(node_features, f32)
    src = np.asarray(edge_index[0]).astype(np.int64)
    dst = np.asarray(edge_index[1]).astype(np.int64)
    B = node_features.shape[0]
    scale = f32(1.0 / np.sqrt(DH))

    bias_full = (np.asarray(pe, f32) @ np.asarray(emb_pe_w, f32)
                 + np.asarray(emb_pe_b, f32) + np.asarray(emb_h_b, f32))
    try:
        h = _embed_device(node_features, np.asarray(emb_h_w, f32), bias_full)
    except Exception:
        import traceback
        traceback.print_exc(file=sys.stderr)
        h = _embed_host(node_features, np.asarray(emb_h_w, f32), bias_full)
    h = h.astype(f32)

    E = src.shape[0]
    for l in range(NLAYERS):
        Q = (h @ wq_w[l] + wq_b[l]).reshape(B, N_NODES, HEADS, DH)
        K = (h @ wk_w[l] + wk_b[l]).reshape(B, N_NODES, HEADS, DH)
        V = (h @ wv_w[l] + wv_b[l]).reshape(B, N_NODES, HEADS, DH)
        score = np.einsum('behd,behd->beh', Q[:, dst], K[:, src],
                          optimize=True) * scale
        attn = _softmax(np.clip(score, -5.0, 5.0), axis=1)  # over all edges
        # dense scatter: E unique (i,j) pairs
        Edense = np.zeros((B, N_NODES * N_NODES, HEADS), f32)
        Edense[:, src * N_NODES + dst, :] = attn
        Edense = Edense.reshape(B, N_NODES, N_NODES, HEADS)
        agg = np.einsum('bijh,bihd->bjhd', Edense, V, optimize=True)
        h_attn = agg.reshape(B, N_NODES, HID) @ wo_w[l] + wo_b[l]
        h = _layer_norm(h + h_attn, ln1_g[l], ln1_b[l])
        ff = np.maximum(h @ lin1_w[l] + lin1_b[l], 0.0) @ lin2_w[l] + lin2_b[l]
        h = _layer_norm(h + ff, ln2_g[l], ln2_b[l])

    pooled = h.mean(axis=1)
    z = np.maximum(pooled @ mlp_w0 + mlp_b0, 0.0)
    z = np.maximum(z @ mlp_w1 + mlp_b1, 0.0)
    return (z @ mlp_w2 + mlp_b2).astype(f32)


# revision 4
# speedup vs baseline: 2.3976x; 2.3976x over previous
import sys

import numpy as np

N_NODES = 19
HID = 128
HEADS = 8
DH = HID // HEADS
NLAYERS = 3
EPS = 1e-5
BATCH = 1024
IN_DIM = 3000
N_CORES = 8


def _layer_norm(x, g, b):
    m = x.mean(axis=-1, keepdims=True)
    v = ((x - m) ** 2).mean(axis=-1, keepdims=True)
    return (x - m) / np.sqrt(v + EPS) * g + b


def _softmax(x, axis):
    x = x - x.max(axis=axis, keepdims=True)
    e = np.exp(x)
    return e / e.sum(axis=axis, keepdims=True)


def _embed_host(node_features, emb_h_w, bias_full):
    # h[b,n,:] = node_features[b,:,n] @ emb_h_w + bias_full[n,:]
    x = np.ascontiguousarray(node_features.transpose(0, 2, 1))
    return x @ emb_h_w + bias_full[None, :, :]


def _embed_device(node_features, emb_h_w, bias_full):
    """Embedding matmul on 8 NeuronCores via bass: shard batch, contract 3000-dim."""
    import sys
    for p in ("/opt/trn_rl_repo",):
        if p not in sys.path:
            sys.path.insert(0, p)
    import concourse.bass as bass
    import concourse.tile as tile
    from concourse import mybir
    from concourse import bass_utils

    B = node_features.shape[0]
    BPC = B // N_CORES          # 128 batches per core
    GB = 16                     # batch group per matmul set
    NG = BPC // GB              # 8 groups
    RCH = 24                    # contraction split: 3000 = 24 * 125
    P = 125

    nc = bass.Bass()
    nf = nc.dram_tensor("nf", [BPC, IN_DIM, N_NODES], mybir.dt.float32,
                        kind="ExternalInput")
    w = nc.dram_tensor("w", [RCH, P, HID], mybir.dt.float32, kind="ExternalInput")
    bias = nc.dram_tensor("bias", [HID, N_NODES], mybir.dt.float32,
                          kind="ExternalInput")
    out = nc.dram_tensor("out", [HID, BPC, N_NODES], mybir.dt.float32,
                         kind="ExternalOutput")

    with tile.TileContext(nc) as tc:
        with tc.tile_pool(name="consts", bufs=1) as consts, \
             tc.tile_pool(name="nfp", bufs=3) as nfp, \
             tc.tile_pool(name="ps", bufs=4, space="PSUM") as ps, \
             tc.tile_pool(name="outp", bufs=3) as outp:
            wt = consts.tile([P, RCH, HID], mybir.dt.float32)
            nc.sync.dma_start(out=wt, in_=w.rearrange("r p m -> p r m"))
            bt = consts.tile([HID, N_NODES], mybir.dt.float32)
            nc.sync.dma_start(out=bt, in_=bias[:, :])
            for g in range(NG):
                nft = nfp.tile([P, GB, RCH, N_NODES], mybir.dt.float32)
                # partition p holds rows [p*24,(p+1)*24) of each [3000,19] slab
                nc.sync.dma_start(
                    out=nft,
                    in_=nf[g * GB:(g + 1) * GB].rearrange("b (p r) n -> p b r n", p=P),
                )
                acc = ps.tile([HID, GB * N_NODES], mybir.dt.float32)
                for r in range(RCH):
                    nc.tensor.matmul(
                        acc,
                        lhsT=wt[:, r, :].bitcast(mybir.dt.float32r),
                        rhs=nft[:, :, r, :].rearrange("p b n -> p (b n)").bitcast(
                            mybir.dt.float32r),
                        start=(r == 0), stop=(r == RCH - 1),
                    )
                ot = outp.tile([HID, GB, N_NODES], mybir.dt.float32)
                nc.vector.tensor_add(
                    ot, acc.rearrange("h (b n) -> h b n", b=GB),
                    bt[:, None, :].broadcast_to([HID, GB, N_NODES]))
                nc.sync.dma_start(out=out[:, g * GB:(g + 1) * GB, :], in_=ot)

    w_re = np.ascontiguousarray(
        emb_h_w.reshape(P, RCH, HID).transpose(1, 0, 2))  # [24,125,128]
    in_maps = []
    for c in range(N_CORES):
        in_maps.append({
            "nf": np.ascontiguousarray(node_features[c * BPC:(c + 1) * BPC]),
            "w": w_re,
            "bias": np.ascontiguousarray(bias_full.T),
        })
    res = bass_utils.run_bass_kernel_spmd(nc, in_maps, core_ids=list(range(N_CORES)))
    hs = [r["out"].transpose(1, 2, 0) for r in res.results]  # [BPC,19,128]
    return np.concatenate(hs, axis=0)


def kernel(node_features, pe, edge_index,
           emb_h_w, emb_h_b, emb_pe_w, emb_pe_b,
           wq_w, wq_b, wk_w, wk_b, wv_w, wv_b, wo_w, wo_b,
           ln1_g, ln1_b, lin1_w, lin1_b, lin2_w, lin2_b, ln2_g, ln2_b,
           mlp_w0, mlp_b0, mlp_w1, mlp_b1, mlp_w2, mlp_b2):
    f32 = np.float32
    node_features = np.asarray(node_features, f32)
    src = np.asarray(edge_index[0]).astype(np.int64)
    dst = np.asarray(edge_index[1]).astype(np.int64)
    B = node_features.shape[0]
    scale = f32(1.0 / np.sqrt(DH))

    bias_full = (np.asarray(pe, f32) @ np.asarray(emb_pe_w, f32)
                 + np.asarray(emb_pe_b, f32) + np.asarray(emb_h_b, f32))
    try:
        h = _embed_device(node_features, np.asarray(emb_h_w, f32), bias_full)
    except Exception:
        import traceback
        traceback.print_exc(file=sys.stderr)
        h = _embed_host(node_features, np.asarray(emb_h_w, f32), bias_full)
    h = h.astype(f32)

    E = src.shape[0]
    for l in range(NLAYERS):
        Q = (h @ wq_w[l] + wq_b[l]).reshape(B, N_NODES, HEADS, DH)
        K = (h @ wk_w[l] + wk_b[l]).reshape(B, N_NODES, HEADS, DH)
        V = (h @ wv_w[l] + wv_b[l]).reshape(B, N_NODES, HEADS, DH)
        score = np.einsum('behd,behd->beh', Q[:, dst], K[:, src],
                          optimize=True) * scale
        attn = _softmax(np.clip(score, -5.0, 5.0), axis=1)  # over all edges
        # dense scatter: E unique (i,j) pairs
        Edense = np.zeros((B, N_NODES * N_NODES, HEADS), f32)
        Edense[:, src * N_NODES + dst, :] = attn
        Edense = Edense.reshape(B, N_NODES, N_NODES, HEADS)
        agg = np.einsum('bijh,bihd->bjhd', Edense, V, optimize=True)
        h_attn = agg.reshape(B, N_NODES, HID) @ wo_w[l] + wo_b[l]
        h = _layer_norm(h + h_attn, ln1_g[l], ln1_b[l])
        ff = np.maximum(h @ lin1_w[l] + lin1_b[l], 0.0) @ lin2_w[l] + lin2_b[l]
        h = _layer_norm(h + ff, ln2_g[l], ln2_b[l])

    pooled = h.mean(axis=1)
    z = np.maximum(pooled @ mlp_w0 + mlp_b0, 0.0)
    z = np.maximum(z @ mlp_w1 + mlp_b1, 0.0)
    return (z @ mlp_w2 + mlp_b2).astype(f32)


# revision 5
# speedup vs baseline: 2.4615x; 1.0266x over previous
import sys

import numpy as np

N_NODES = 19
HID = 128
HEADS = 8
DH = HID // HEADS
NLAYERS = 3
EPS = 1e-5
BATCH = 1024
IN_DIM = 3000
N_CORES = 8


def _layer_norm(x, g, b):
    m = x.mean(axis=-1, keepdims=True)
    v = ((x - m) ** 2).mean(axis=-1, keepdims=True)
    return (x - m) / np.sqrt(v + EPS) * g + b


def _softmax(x, axis):
    x = x - x.max(axis=axis, keepdims=True)
    e = np.exp(x)
    return e / e.sum(axis=axis, keepdims=True)


def _embed_host(node_features, emb_h_w, bias_full):
    # h[b,n,:] = node_features[b,:,n] @ emb_h_w + bias_full[n,:]
    x = np.ascontiguousarray(node_features.transpose(0, 2, 1))
    return x @ emb_h_w + bias_full[None, :, :]


def _embed_device(node_features, emb_h_w, bias_full):
    """Embedding matmul on 8 NeuronCores via bass: shard batch, contract 3000-dim."""
    import sys
    for p in ("/opt/trn_rl_repo",):
        if p not in sys.path:
            sys.path.insert(0, p)
    import concourse.bass as bass
    import concourse.tile as tile
    from concourse import mybir
    from concourse import bass_utils

    B = node_features.shape[0]
    BPC = B // N_CORES          # 128 batches per core
    GB = 16                     # batch group per matmul set
    NG = BPC // GB              # 8 groups
    RCH = 24                    # contraction split: 3000 = 24 * 125
    P = 125

    nc = bass.Bass()
    nf = nc.dram_tensor("nf", [BPC, IN_DIM, N_NODES], mybir.dt.float32,
                        kind="ExternalInput")
    w = nc.dram_tensor("w", [RCH, P, HID], mybir.dt.float32, kind="ExternalInput")
    bias = nc.dram_tensor("bias", [HID, N_NODES], mybir.dt.float32,
                          kind="ExternalInput")
    out = nc.dram_tensor("out", [HID, BPC, N_NODES], mybir.dt.float32,
                         kind="ExternalOutput")

    with tile.TileContext(nc) as tc:
        with tc.tile_pool(name="consts", bufs=1) as consts, \
             tc.tile_pool(name="nfp", bufs=3) as nfp, \
             tc.tile_pool(name="ps", bufs=4, space="PSUM") as ps, \
             tc.tile_pool(name="outp", bufs=3) as outp:
            wt = consts.tile([P, RCH, HID], mybir.dt.float32)
            nc.sync.dma_start(out=wt, in_=w.rearrange("r p m -> p r m"))
            bt = consts.tile([HID, N_NODES], mybir.dt.float32)
            nc.sync.dma_start(out=bt, in_=bias[:, :])
            for g in range(NG):
                nft = nfp.tile([P, GB, RCH, N_NODES], mybir.dt.float32)
                # partition p holds rows [p*24,(p+1)*24) of each [3000,19] slab
                nc.sync.dma_start(
                    out=nft,
                    in_=nf[g * GB:(g + 1) * GB].rearrange("b (p r) n -> p b r n", p=P),
                )
                acc = ps.tile([HID, GB, N_NODES], mybir.dt.float32)
                for r in range(RCH):
                    nc.tensor.matmul(
                        acc,
                        lhsT=wt[:, r, :].bitcast(mybir.dt.float32r),
                        rhs=nft[:, :, r, :].bitcast(mybir.dt.float32r),
                        start=(r == 0), stop=(r == RCH - 1),
                    )
                ot = outp.tile([HID, GB, N_NODES], mybir.dt.float32)
                nc.vector.tensor_add(
                    ot, acc,
                    bt[:, None, :].broadcast_to([HID, GB, N_NODES]))
                nc.sync.dma_start(out=out[:, g * GB:(g + 1) * GB, :], in_=ot)

    w_re = np.ascontiguousarray(
        emb_h_w.reshape(P, RCH, HID).transpose(1, 0, 2))  # [24,125,128]
    in_maps = []
    for c in range(N_CORES):
        in_maps.append({
            "nf": np.ascontiguousarray(node_features[c * BPC:(c + 1) * BPC]),
            "w": w_re,
            "bias": np.ascontiguousarray(bias_full.T),
        })
    res = bass_utils.run_bass_kernel_spmd(nc, in_maps, core_ids=list(range(N_CORES)))
    hs = [r["out"].transpose(1, 2, 0) for r in res.results]  # [BPC,19,128]
    return np.concatenate(hs, axis=0)


def kernel(node_features, pe, edge_index,
           emb_h_w, emb_h_b, emb_pe_w, emb_pe_b,
           wq_w, wq_b, wk_w, wk_b, wv_w, wv_b, wo_w, wo_b,
           ln1_g, ln1_b, lin1_w, lin1_b, lin2_w, lin2_b, ln2_g, ln2_b,
           mlp_w0, mlp_b0, mlp_w1, mlp_b1, mlp_w2, mlp_b2):
    f32 = np.float32
    node_features = np.asarray(node_features, f32)
    src = np.asarray(edge_index[0]).astype(np.int64)
    dst = np.asarray(edge_index[1]).astype(np.int64)
    B = node_features.shape[0]
    scale = f32(1.0 / np.sqrt(DH))

    bias_full = (np.asarray(pe, f32) @ np.asarray(emb_pe_w, f32)
                 + np.asarray(emb_pe_b, f32) + np.asarray(emb_h_b, f32))
    try:
        h = _embed_device(node_features, np.asarray(emb_h_w, f32), bias_full)
    except Exception:
        import traceback
        traceback.print_exc(file=sys.stderr)
        h = _embed_host(node_features, np.asarray(emb_h_w, f32), bias_full)
    h = h.astype(f32)

    E = src.shape[0]
    for l in range(NLAYERS):
        Q = (h @ wq_w[l] + wq_b[l]).reshape(B, N_NODES, HEADS, DH)
        K = (h @ wk_w[l] + wk_b[l]).reshape(B, N_NODES, HEADS, DH)
        V = (h @ wv_w[l] + wv_b[l]).reshape(B, N_NODES, HEADS, DH)
        score = np.einsum('behd,behd->beh', Q[:, dst], K[:, src],
                          optimize=True) * scale
        attn = _softmax(np.clip(score, -5.0, 5.0), axis=1)  # over all edges
        # dense scatter: E unique (i,j) pairs
        Edense = np.zeros((B, N_NODES * N_NODES, HEADS), f32)
        Edense[:, src * N_NODES + dst, :] = attn
        Edense = Edense.reshape(B, N_NODES, N_NODES, HEADS)
        agg = np.einsum('bijh,bihd->bjhd', Edense, V, optimize=True)
        h_attn = agg.reshape(B, N_NODES, HID) @ wo_w[l] + wo_b[l]
        h = _layer_norm(h + h_attn, ln1_g[l], ln1_b[l])
        ff = np.maximum(h @ lin1_w[l] + lin1_b[l], 0.0) @ lin2_w[l] + lin2_b[l]
        h = _layer_norm(h + ff, ln2_g[l], ln2_b[l])

    pooled = h.mean(axis=1)
    z = np.maximum(pooled @ mlp_w0 + mlp_b0, 0.0)
    z = np.maximum(z @ mlp_w1 + mlp_b1, 0.0)
    return (z @ mlp_w2 + mlp_b2).astype(f32)


# revision 8
# speedup vs baseline: 3.0706x; 1.2475x over previous
import sys

import numpy as np

N_NODES = 19
HID = 128
HEADS = 8
DH = HID // HEADS
NLAYERS = 3
EPS = 1e-5
BATCH = 1024
IN_DIM = 3000
N_CORES = 8


def _layer_norm(x, g, b):
    m = x.mean(axis=-1, keepdims=True)
    v = ((x - m) ** 2).mean(axis=-1, keepdims=True)
    return (x - m) / np.sqrt(v + EPS) * g + b


def _softmax(x, axis):
    x = x - x.max(axis=axis, keepdims=True)
    e = np.exp(x)
    return e / e.sum(axis=axis, keepdims=True)


def _embed_host(node_features, emb_h_w, bias_full):
    # h[b,n,:] = node_features[b,:,n] @ emb_h_w + bias_full[n,:]
    x = np.ascontiguousarray(node_features.transpose(0, 2, 1))
    return x @ emb_h_w + bias_full[None, :, :]


def _embed_device(node_features, emb_h_w, bias_full):
    """Embedding matmul on 8 NeuronCores via bass: shard batch, contract 3000-dim."""
    import sys
    for p in ("/opt/trn_rl_repo",):
        if p not in sys.path:
            sys.path.insert(0, p)
    import concourse.bass as bass
    import concourse.tile as tile
    from concourse import mybir
    from concourse import bass_utils

    B = node_features.shape[0]
    BPC = B // N_CORES          # 128 batches per core
    GB = 16                     # batch group per matmul set
    NG = BPC // GB              # 8 groups
    RCH = 24                    # contraction split: 3000 = 24 * 125
    P = 125

    nc = bass.Bass()
    nf = nc.dram_tensor("nf", [BPC, IN_DIM, N_NODES], mybir.dt.float32,
                        kind="ExternalInput")
    w = nc.dram_tensor("w", [RCH, P, HID], mybir.dt.float32, kind="ExternalInput")
    out = nc.dram_tensor("out", [HID, BPC, N_NODES], mybir.dt.float32,
                         kind="ExternalOutput")

    with tile.TileContext(nc) as tc:
        with tc.tile_pool(name="consts", bufs=1) as consts, \
             tc.tile_pool(name="nfp", bufs=3) as nfp, \
             tc.tile_pool(name="ps", bufs=4, space="PSUM") as ps, \
             tc.tile_pool(name="outp", bufs=3) as outp:
            wts = []
            for r in range(RCH):
                wr = consts.tile([P, HID], mybir.dt.bfloat16, tag=f"w{r}")
                nc.gpsimd.dma_start(out=wr, in_=w[r])
                wts.append(wr)
            for g in range(NG):
                nft = nfp.tile([P, GB, RCH, N_NODES], mybir.dt.bfloat16)
                # partition p holds rows [p*24,(p+1)*24) of each [3000,19] slab
                nc.gpsimd.dma_start(
                    out=nft,
                    in_=nf[g * GB:(g + 1) * GB].rearrange("b (p r) n -> p b r n", p=P),
                )
                acc = ps.tile([HID, GB, N_NODES], mybir.dt.float32)
                for r in range(RCH):
                    nc.tensor.matmul(
                        acc,
                        lhsT=wts[r],
                        rhs=nft[:, :, r, :],
                        start=(r == 0), stop=(r == RCH - 1),
                    )
                ot = outp.tile([HID, GB, N_NODES], mybir.dt.float32)
                nc.vector.tensor_copy(ot, acc)
                nc.sync.dma_start(out=out[:, g * GB:(g + 1) * GB, :], in_=ot)

    w_re = np.ascontiguousarray(
        emb_h_w.reshape(P, RCH, HID).transpose(1, 0, 2))  # [24,125,128]
    in_maps = []
    for c in range(N_CORES):
        in_maps.append({
            "nf": np.ascontiguousarray(node_features[c * BPC:(c + 1) * BPC]),
            "w": w_re,
        })
    res = bass_utils.run_bass_kernel_spmd(nc, in_maps, core_ids=list(range(N_CORES)))
    hs = [r["out"].transpose(1, 2, 0) for r in res.results]  # [BPC,19,128]
    return np.concatenate(hs, axis=0) + bias_full[None, :, :]


def kernel(node_features, pe, edge_index,
           emb_h_w, emb_h_b, emb_pe_w, emb_pe_b,
           wq_w, wq_b, wk_w, wk_b, wv_w, wv_b, wo_w, wo_b,
           ln1_g, ln1_b, lin1_w, lin1_b, lin2_w, lin2_b, ln2_g, ln2_b,
           mlp_w0, mlp_b0, mlp_w1, mlp_b1, mlp_w2, mlp_b2):
    f32 = np.float32
    node_features = np.asarray(node_features, f32)
    src = np.asarray(edge_index[0]).astype(np.int64)
    dst = np.asarray(edge_index[1]).astype(np.int64)
    B = node_features.shape[0]
    scale = f32(1.0 / np.sqrt(DH))

    bias_full = (np.asarray(pe, f32) @ np.asarray(emb_pe_w, f32)
                 + np.asarray(emb_pe_b, f32) + np.asarray(emb_h_b, f32))
    try:
        h = _embed_device(node_features, np.asarray(emb_h_w, f32), bias_full)
    except Exception:
        import traceback
        traceback.print_exc(file=sys.stderr)
        h = _embed_host(node_features, np.asarray(emb_h_w, f32), bias_full)
    h = h.astype(f32)

    E = src.shape[0]
    for l in range(NLAYERS):
        Q = (h @ wq_w[l] + wq_b[l]).reshape(B, N_NODES, HEADS, DH)
        K = (h @ wk_w[l] + wk_b[l]).reshape(B, N_NODES, HEADS, DH)
        V = (h @ wv_w[l] + wv_b[l]).reshape(B, N_NODES, HEADS, DH)
        score = np.einsum('behd,behd->beh', Q[:, dst], K[:, src],
                          optimize=True) * scale
        attn = _softmax(np.clip(score, -5.0, 5.0), axis=1)  # over all edges
        # dense scatter: E unique (i,j) pairs
        Edense = np.zeros((B, N_NODES * N_NODES, HEADS), f32)
        Edense[:, src * N_NODES + dst, :] = attn
        Edense = Edense.reshape(B, N_NODES, N_NODES, HEADS)
        agg = np.einsum('bijh,bihd->bjhd', Edense, V, optimize=True)
        h_attn = agg.reshape(B, N_NODES, HID) @ wo_w[l] + wo_b[l]
        h = _layer_norm(h + h_attn, ln1_g[l], ln1_b[l])
        ff = np.maximum(h @ lin1_w[l] + lin1_b[l], 0.0) @ lin2_w[l] + lin2_b[l]
        h = _layer_norm(h + ff, ln2_g[l], ln2_b[l])

    pooled = h.mean(axis=1)
    z = np.maximum(pooled @ mlp_w0 + mlp_b0, 0.0)
    z = np.maximum(z @ mlp_w1 + mlp_b1, 0.0)
    return (z @ mlp_w2 + mlp_b2).astype(f32)
